# revision 1
# baseline (speedup 1.0000x reference)
"""DeepseekV2 MLA attention (B=2, S=2048, HID=4096, H=32, QK=192, VD=128)
on 8 trn2 NeuronCores.

Sharding: batch x sequence-quarter (8 cores = 2 batches x 4 query quarters).
Each core computes the full kv path for its batch (replicated within the
batch's 4 cores) and full-horizon masked causal attention for its 512
query tokens, plus o_proj for those tokens. No collectives; host only
slices/permutes inputs and concatenates outputs.

All on-chip compute is feature-major ([feature partitions, token free]) so
the whole chain (q_a -> rmsnorm -> q_b -> rope -> scores^T -> softmax ->
PV^T -> o_proj) needs zero on-device transposes. Matmuls run in float32r
(1 cycle/row at free>=256, ~2e-4 relative rounding).
"""
import sys

sys.path.insert(0, "/opt/trn_rl_repo")

import numpy as np
import concourse.bass as bass  # noqa: F401  (engine classes referenced via nc)
from concourse import bacc
import concourse.mybir as mybir
import concourse.tile as tile
from concourse.bass_utils import run_bass_kernel_spmd

# ---- problem constants (hardcoded per contract) ----
B, S, HID = 2, 2048, 4096
H, NOPE, ROPE, VD = 32, 128, 64, 128
QK = NOPE + ROPE          # 192
QLR, KVLR = 1536, 512
EPS = 1e-6
SCALE = QK ** -0.5

P = 128                   # partitions
FREE = 512                # query tokens per core
TOK = 2048                # key tokens per core (= S of its batch)
HC = HID // P             # 32 hid chunks
QC = QLR // P             # 12 q-latent chunks
KC = KVLR // P            # 4 kv-latent chunks
NKB = TOK // P            # 16 key blocks
NEG = np.float32(-1e32)   # additive mask value (pre-exp-scale)

fr = mybir.dt.float32r
f32 = mybir.dt.float32

_CACHED = {}

KNOBS = dict(ps_s_bufs=5, ps_o_bufs=2, ps_den_bufs=1, probs_bufs=4,
             split_half_psum=False, nheads=H, do_phase_c=True, do_attn=True)


def _build_nc():
    key = tuple(sorted(KNOBS.items()))
    if key in _CACHED:
        return _CACHED[key]
    nc = bacc.Bacc("TRN2", target_bir_lowering=False, debug=False, num_devices=8)
    d = lambda name, shape: nc.dram_tensor(name, shape, f32, kind="ExternalInput").ap()
    kh = d("kh_T", [HID, TOK])
    qh = d("qh_T", [HID, FREE])
    wqa = d("wq_a_T", [HID, QLR])
    wqb = d("wq_b_T", [QLR, H * QK])
    wkva = d("wkv_a_T", [HID, KVLR + ROPE])
    wkvb = d("wkv_b_T", [KVLR, H * (NOPE + VD)])
    wo = d("wo_T", [H * VD, HID])
    cc2q_in = d("cc2q", [ROPE, FREE])
    ss2q_in = d("ss2q", [ROPE, FREE])
    cc2k_in = d("cc2k", [ROPE, TOK])
    ss2k_in = d("ss2k", [ROPE, TOK])
    perm_in = d("perm64", [ROPE, ROPE])
    mtril = d("mask_tril", [256, 256])
    mcolE = d("mask_col_e", [P, 8])
    mcolL = d("mask_col_l", [P, NKB])
    ones_in = d("ones_in", [P, P])
    outT = nc.dram_tensor("out_T", [HID, FREE], f32, kind="ExternalOutput").ap()
    attn_dram = nc.dram_tensor("attn_scratch", [H * VD, FREE], f32).ap()

    wkvb4 = wkvb.rearrange("(lc p) (hh c) -> p lc hh c", p=P, c=NOPE + VD)

    with tile.TileContext(nc) as tc:
        with tc.tile_pool(name="persist", bufs=1) as persist:
            q_a_n = persist.tile([P, QC, FREE], fr)        # q latent (normed in place)
            k_lat = persist.tile([P, KC, TOK], fr)         # kv latent (normed in place)
            k_rot_n = persist.tile([ROPE, TOK], fr)        # shared rope'd key
            tril_sb = persist.tile([P, 2, 256], f32)
            mask_col_e = persist.tile([P, 8], f32)
            mask_col_l = persist.tile([P, NKB], f32)
            cc2q = persist.tile([ROPE, FREE], f32)
            ss2q = persist.tile([ROPE, FREE], f32)
            perm64 = persist.tile([ROPE, ROPE], fr)
            ones128 = persist.tile([P, 1], fr)
            ones1 = persist.tile([1, P], fr)
            eps_t = persist.tile([1, 1], f32)
            nc.vector.memset(eps_t[:], EPS)

            nc.sync.dma_start(tril_sb[:], mtril.rearrange("(kb p) q -> p kb q", p=P))
            nc.sync.dma_start(mask_col_e[:], mcolE)
            nc.sync.dma_start(mask_col_l[:], mcolL)
            nc.sync.dma_start(cc2q[:], cc2q_in)
            nc.sync.dma_start(ss2q[:], ss2q_in)
            nc.sync.dma_start(perm64[:], perm_in.bitcast(fr))
            nc.sync.dma_start(ones128[:], ones_in[:, 0:1].bitcast(fr))
            nc.sync.dma_start(ones1[:], ones_in[0:1, :].bitcast(fr))

            # ================= Phase A: latents =================
            with tc.tile_pool(name="pa", bufs=2) as pa, \
                 tc.tile_pool(name="pa1", bufs=1) as pa1, \
                 tc.tile_pool(name="pars", bufs=4) as pars, \
                 tc.tile_pool(name="paw", bufs=6) as paw, \
                 tc.tile_pool(name="pasq", bufs=3) as pasq, \
                 tc.tile_pool(name="pa_ps", bufs=6, space="PSUM") as paps, \
                 tc.tile_pool(name="sums_ps", bufs=2, space="PSUM") as sums_ps:
                cc2k = pa1.tile([ROPE, TOK], f32)
                ss2k = pa1.tile([ROPE, TOK], f32)
                nc.sync.dma_start(cc2k[:], cc2k_in)
                nc.sync.dma_start(ss2k[:], ss2k_in)
                k_rot_raw = pa1.tile([ROPE, TOK], f32)
                k_partner = pa1.tile([ROPE, TOK], f32)

                sums_q = sums_ps.tile([1, FREE], f32, tag="sums")

                def gemm_pass(src_ap, tcol, grp, sums_tile, t):
                    # grp: list of ("q"|"k"|"rot", i); rhs from src_ap
                    # columns [tcol*FREE, (tcol+1)*FREE)
                    psums = {}
                    for kind, i in grp:
                        rows = ROPE if kind == "rot" else P
                        psums[(kind, i)] = paps.tile(
                            [rows, FREE], f32, tag="pa",
                            name=f"pa_{kind}{i}_t{t}")
                    NQ = 4
                    CQH = HC // NQ
                    for qr in range(NQ):
                        kht = pa.tile([P, CQH, FREE], fr, tag="kh",
                                      name=f"kh_{t}_{qr}")
                        nc.sync.dma_start(
                            kht[:],
                            src_ap[qr * 1024:(qr + 1) * 1024,
                                   tcol * FREE:(tcol + 1) * FREE]
                            .rearrange("(hc p) n -> p hc n", p=P).bitcast(fr))
                        for kind, i in grp:
                            if kind == "q":
                                wsrc = wqa[qr * 1024:(qr + 1) * 1024,
                                           i * P:(i + 1) * P]
                                cols = P
                            elif kind == "k":
                                wsrc = wkva[qr * 1024:(qr + 1) * 1024,
                                            i * P:(i + 1) * P]
                                cols = P
                            else:
                                wsrc = wkva[qr * 1024:(qr + 1) * 1024,
                                            KVLR:KVLR + ROPE]
                                cols = ROPE
                            wt = paw.tile([P, CQH, P], fr, tag="w",
                                          name=f"w_{kind}{i}_{t}_{qr}")
                            nc.sync.dma_start(
                                wt[:, :, :cols],
                                wsrc.rearrange("(hc p) m -> p hc m", p=P)
                                .bitcast(fr))
                            ps_ = psums[(kind, i)]
                            for hc in range(CQH):
                                nc.tensor.matmul(
                                    ps_[:], wt[:, hc, :cols],
                                    kht[:, hc, :],
                                    start=(qr == 0 and hc == 0),
                                    stop=(qr == NQ - 1 and hc == CQH - 1))
                    for kind, i in grp:
                        ps_ = psums[(kind, i)]
                        if kind == "q":
                            sq = pasq.tile([P, FREE], fr, tag="sq",
                                           name=f"sq_q{i}")
                            nc.scalar.activation(
                                sq[:], ps_[:],
                                mybir.ActivationFunctionType.Square)
                            nc.tensor.matmul(sums_tile[:], ones128[:], sq[:],
                                             start=(i == 0),
                                             stop=(i == QC - 1))
                            nc.vector.tensor_copy(q_a_n[:, i, :], ps_[:])
                        elif kind == "k":
                            sq = pasq.tile([P, FREE], fr, tag="sq",
                                           name=f"sq_k{i}_{t}")
                            nc.scalar.activation(
                                sq[:], ps_[:],
                                mybir.ActivationFunctionType.Square)
                            nc.tensor.matmul(sums_tile[:], ones128[:], sq[:],
                                             start=(i == 0),
                                             stop=(i == KC - 1))
                            nc.vector.tensor_copy(
                                k_lat[:, i, t * FREE:(t + 1) * FREE], ps_[:])
                        else:
                            nc.vector.tensor_copy(
                                k_rot_raw[:, t * FREE:(t + 1) * FREE], ps_[:])

                # q pass: 12 obs over qh (two groups of 6)
                qobs = [("q", i) for i in range(QC)]
                gemm_pass(qh, 0, qobs[:6], sums_q, 8)
                gemm_pass(qh, 0, qobs[6:], sums_q, 9)
                # kv passes: per token chunk over kh
                kobs = [("k", i) for i in range(KC)] + [("rot", 0)]
                for t in range(4):
                    sums_k = sums_ps.tile([1, FREE], f32, tag="sums",
                                          name=f"sums_k{t}")
                    gemm_pass(kh, t, kobs, sums_k, t)
                    rsk_sq = pars.tile([1, FREE], f32, tag="rsrowsq",
                                       name=f"rsk_sq{t}")
                    nc.scalar.activation(rsk_sq[:], sums_k[:],
                                         mybir.ActivationFunctionType.Sqrt,
                                         bias=eps_t[:], scale=1.0 / KVLR)
                    rsk_row = pars.tile([1, FREE], fr, tag="rsrow",
                                        name=f"rsk_row{t}")
                    with nc.allow_low_precision(reason="fr rounding of rms scale is ~2e-4"):
                        nc.vector.reciprocal(rsk_row[:], rsk_sq[:])
                    psb = paps.tile([P, FREE], f32, tag="pa", name=f"bc_k{t}")
                    nc.tensor.matmul(psb[:], ones1[:], rsk_row[:],
                                     start=True, stop=True)
                    rskb = pars.tile([P, FREE], f32, tag="rskb",
                                     name=f"rskb{t}")
                    nc.vector.tensor_copy(rskb[:], psb[:])
                    for i in range(KC):
                        nc.vector.tensor_tensor(
                            out=k_lat[:, i, t * FREE:(t + 1) * FREE],
                            in0=k_lat[:, i, t * FREE:(t + 1) * FREE],
                            in1=rskb[:], op=mybir.AluOpType.mult)

                # rs_q -> broadcast -> normalize q_a in place
                rsq_sq = pars.tile([1, FREE], f32, tag="rsrowsq")
                nc.scalar.activation(rsq_sq[:], sums_q[:],
                                     mybir.ActivationFunctionType.Sqrt,
                                     bias=eps_t[:], scale=1.0 / QLR)
                rsq_row = pars.tile([1, FREE], fr, tag="rsrow")
                with nc.allow_low_precision(reason="fr rounding of rms scale is ~2e-4"):
                    nc.vector.reciprocal(rsq_row[:], rsq_sq[:])
                psbq = paps.tile([P, FREE], f32, tag="pa", name="bc_q")
                nc.tensor.matmul(psbq[:], ones1[:], rsq_row[:],
                                 start=True, stop=True)
                rsqb = pars.tile([P, FREE], f32, tag="rskb")
                nc.vector.tensor_copy(rsqb[:], psbq[:])
                for i in range(QC):
                    nc.vector.tensor_tensor(
                        out=q_a_n[:, i, :], in0=q_a_n[:, i, :], in1=rsqb[:],
                        op=mybir.AluOpType.mult)

                # rope on shared key rot (no norm on k_rot), chunked.
                # partner[p] = raw[(p+32)%64] via sbuf->sbuf DMA (crosses
                # partitions); out = raw*cc2 + partner*ss2.
                HR = ROPE // 2
                nc.sync.dma_start(k_partner[:HR, :], k_rot_raw[HR:, :])
                nc.sync.dma_start(k_partner[HR:, :], k_rot_raw[:HR, :])
                for t in range(4):
                    sl = slice(t * FREE, (t + 1) * FREE)
                    t1 = pars.tile([ROPE, FREE], f32, tag="ropetmp", name=f"ra{t}")
                    t2 = pars.tile([ROPE, FREE], f32, tag="ropetmp", name=f"rb{t}")
                    nc.vector.tensor_tensor(out=t1[:], in0=k_rot_raw[:, sl],
                                            in1=cc2k[:, sl],
                                            op=mybir.AluOpType.mult)
                    nc.vector.tensor_tensor(out=t2[:], in0=k_partner[:, sl],
                                            in1=ss2k[:, sl],
                                            op=mybir.AluOpType.mult)
                    nc.vector.tensor_tensor(out=k_rot_n[:, sl], in0=t1[:],
                                            in1=t2[:],
                                            op=mybir.AluOpType.add)

            # ================= Phase B: heads =================
            with tc.tile_pool(name="hb", bufs=1) as hb, \
                 tc.tile_pool(name="hb2", bufs=2) as hb2, \
                 tc.tile_pool(name="hb4", bufs=KNOBS["probs_bufs"]) as hb4, \
                 tc.tile_pool(name="wb", bufs=2) as wb, \
                 tc.tile_pool(name="ps_s", bufs=KNOBS["ps_s_bufs"], space="PSUM") as ps_s, \
                 tc.tile_pool(name="ps_o", bufs=KNOBS["ps_o_bufs"], space="PSUM") as ps_o, \
                 tc.tile_pool(name="ps_den", bufs=KNOBS["ps_den_bufs"], space="PSUM") as ps_den:
                HR = ROPE // 2
                for h in range(KNOBS["nheads"]):
                    g, gi = divmod(h, 4)
                    if gi == 0:
                        # v for 4-head group, token-major: [tok_part, kb, 4*VD]
                        vg = hb.tile([P, NKB, 4 * VD], fr, tag="vg")
                        wv = wb.tile([P, KC, 4, VD], fr, tag="wv")
                        for lc in range(KC):
                            nc.sync.dma_start(
                                wv[:, lc],
                                wkvb4[:, lc, 4 * g:4 * g + 4, NOPE:].bitcast(fr))
                        for tb in range(NKB):
                            psv = ps_s.tile([P, 4 * VD], f32, tag="s",
                                            name=f"v{g}_{tb}")
                            for lc in range(KC):
                                nc.tensor.matmul(
                                    psv[:],
                                    k_lat[:, lc, tb * P:(tb + 1) * P],
                                    wv[:, lc, :, :].rearrange("p a b -> p (a b)"),
                                    start=(lc == 0), stop=(lc == KC - 1))
                            nc.vector.tensor_copy(vg[:, tb, :], psv[:])
                    # k_pass for this head, feature-major [NOPE, tok]
                    kh_sb = hb.tile([NOPE, KC, FREE], fr, tag="khead")
                    wk = wb.tile([P, KC, NOPE], fr, tag="wk")
                    nc.sync.dma_start(
                        wk[:], wkvb4[:, :, h, :NOPE].bitcast(fr))
                    for tt in range(4):
                        psk = ps_s.tile([NOPE, FREE], f32, tag="s",
                                        name=f"k{h}_{tt}")
                        for lc in range(KC):
                            nc.tensor.matmul(
                                psk[:], wk[:, lc, :],
                                k_lat[:, lc, tt * FREE:(tt + 1) * FREE],
                                start=(lc == 0), stop=(lc == KC - 1))
                        nc.vector.tensor_copy(kh_sb[:, tt, :], psk[:])
                    # q for this head
                    wqh = wb.tile([P, QC, QK], fr, tag="wqh")
                    nc.sync.dma_start(
                        wqh[:],
                        wqb[:, h * QK:(h + 1) * QK]
                        .rearrange("(lc p) m -> p lc m", p=P).bitcast(fr))
                    psqp = ps_s.tile([NOPE, FREE], f32, tag="s", name=f"qp{h}")
                    psqr = ps_s.tile([ROPE, FREE], f32, tag="s", name=f"qr{h}")
                    for lc in range(QC):
                        nc.tensor.matmul(psqp[:], wqh[:, lc, :NOPE],
                                         q_a_n[:, lc, :],
                                         start=(lc == 0), stop=(lc == QC - 1))
                    for lc in range(QC):
                        nc.tensor.matmul(psqr[:], wqh[:, lc, NOPE:],
                                         q_a_n[:, lc, :],
                                         start=(lc == 0), stop=(lc == QC - 1))
                    q_pass = hb2.tile([NOPE, FREE], fr, tag="qpass")
                    nc.vector.tensor_copy(q_pass[:], psqp[:])
                    qr_sb = hb2.tile([ROPE, FREE], fr, tag="qrsb")
                    nc.vector.tensor_copy(qr_sb[:], psqr[:])
                    pperm = ps_s.tile([ROPE, FREE], f32, tag="s",
                                      name=f"qperm{h}")
                    nc.tensor.matmul(pperm[:], perm64[:], qr_sb[:],
                                     start=True, stop=True)
                    q_rot = hb2.tile([ROPE, FREE], fr, tag="qrot")
                    r1 = hb4.tile([ROPE, FREE], f32, tag="ropetmp")
                    r2 = hb4.tile([ROPE, FREE], f32, tag="ropetmp")
                    nc.vector.tensor_tensor(out=r1[:], in0=qr_sb[:], in1=cc2q[:],
                                            op=mybir.AluOpType.mult)
                    nc.vector.tensor_tensor(out=r2[:], in0=pperm[:], in1=ss2q[:],
                                            op=mybir.AluOpType.mult)
                    nc.vector.tensor_tensor(out=q_rot[:], in0=r1[:], in1=r2[:],
                                            op=mybir.AluOpType.add)
                    # attention: Early half (cols 0:256, kb 0..7) and
                    # Late half (cols 256:512, kb 0..15). Causality by
                    # construction: E's visible keys all lie in kb 0..7.
                    if not KNOBS["do_attn"]:
                        attn_sb0 = hb2.tile([VD, FREE], f32, tag="attn")
                        nc.vector.tensor_copy(attn_sb0[:], q_pass[:].bitcast(f32))
                        nc.sync.dma_start(attn_dram[h * VD:(h + 1) * VD, :], attn_sb0[:])
                        continue
                    if not KNOBS["split_half_psum"]:
                        pso_all = ps_o.tile([VD, FREE], f32, tag="o",
                                            name=f"o{h}")
                        psd_all = ps_den.tile([1, FREE], f32, tag="den",
                                              name=f"d{h}")
                    for half, nkb, q0 in (("E", 8, 0), ("L", NKB, 256)):
                        if KNOBS["split_half_psum"]:
                            pso_h = ps_o.tile([VD, 256], f32, tag="o",
                                              name=f"o{h}{half}")
                            psd_h = ps_den.tile([1, 256], f32, tag="den",
                                                name=f"d{h}{half}")
                            if half == "E":
                                psoE, psdE = pso_h, psd_h
                            else:
                                psoL, psdL = pso_h, psd_h
                        qsl = slice(q0, q0 + 256)
                        for kb in range(nkb):
                            tt, off = divmod(kb, 4)
                            pss = ps_s.tile([P, 256], f32, tag="s",
                                            name=f"s{h}{half}{kb}")
                            nc.tensor.matmul(pss[:],
                                             kh_sb[:, tt, off * P:(off + 1) * P],
                                             q_pass[:, qsl],
                                             start=True, stop=False)
                            nc.tensor.matmul(pss[:],
                                             k_rot_n[:, kb * P:(kb + 1) * P],
                                             q_rot[:, qsl],
                                             start=False, stop=True)
                            if half == "E" and kb < 2:
                                nc.vector.tensor_tensor(
                                    out=pss[:], in0=pss[:],
                                    in1=tril_sb[:, kb, :],
                                    op=mybir.AluOpType.add)
                            elif half == "L" and kb >= NKB - 2:
                                nc.vector.tensor_tensor(
                                    out=pss[:], in0=pss[:],
                                    in1=tril_sb[:, kb - (NKB - 2), :],
                                    op=mybir.AluOpType.add)
                            else:
                                mcol_t = mask_col_e if half == "E" else mask_col_l
                                nc.vector.tensor_scalar(
                                    out=pss[:], in0=pss[:],
                                    scalar1=mcol_t[:, kb:kb + 1], scalar2=None,
                                    op0=mybir.AluOpType.add)
                            probs = hb4.tile([P, 256], fr, tag="probs")
                            nc.scalar.activation(
                                probs[:], pss[:],
                                mybir.ActivationFunctionType.Exp, scale=SCALE)
                            psd_t = psd_h if KNOBS["split_half_psum"] else psd_all[:, qsl]
                            pso_t = pso_h if KNOBS["split_half_psum"] else pso_all[:, qsl]
                            nc.tensor.matmul(psd_t, ones128[:], probs[:],
                                             start=(kb == 0),
                                             stop=(kb == nkb - 1))
                            nc.tensor.matmul(pso_t,
                                             vg[:, kb, gi * VD:(gi + 1) * VD],
                                             probs[:],
                                             start=(kb == 0),
                                             stop=(kb == nkb - 1))
                    # normalize and store
                    rec = hb2.tile([1, FREE], fr, tag="rec")
                    attn_sb = hb2.tile([VD, FREE], f32, tag="attn")
                    if KNOBS["split_half_psum"]:
                        with nc.allow_low_precision(reason="fr denom rounding ~2e-4"):
                            nc.vector.reciprocal(rec[:, 0:256], psdE[:])
                            nc.vector.reciprocal(rec[:, 256:], psdL[:])
                    else:
                        with nc.allow_low_precision(reason="fr denom rounding ~2e-4"):
                            nc.vector.reciprocal(rec[:], psd_all[:])
                    psb2 = ps_s.tile([P, FREE], f32, tag="s", name=f"bc{h}")
                    nc.tensor.matmul(psb2[:], ones1[:], rec[:],
                                     start=True, stop=True)
                    recb = hb2.tile([P, FREE], f32, tag="recb")
                    nc.vector.tensor_copy(recb[:], psb2[:])
                    if KNOBS["split_half_psum"]:
                        nc.vector.tensor_tensor(out=attn_sb[:, 0:256],
                                                in0=psoE[:], in1=recb[:VD, 0:256],
                                                op=mybir.AluOpType.mult)
                        nc.vector.tensor_tensor(out=attn_sb[:, 256:],
                                                in0=psoL[:], in1=recb[:VD, 256:],
                                                op=mybir.AluOpType.mult)
                    else:
                        nc.vector.tensor_tensor(out=attn_sb[:], in0=pso_all[:],
                                                in1=recb[:VD, :],
                                                op=mybir.AluOpType.mult)
                    nc.sync.dma_start(attn_dram[h * VD:(h + 1) * VD, :],
                                      attn_sb[:])

        # ================= Phase C: o_proj =================
        with tc.tile_pool(name="pc", bufs=3) as pc, \
             tc.tile_pool(name="pc_ps", bufs=8, space="PSUM") as pcps:
            if not KNOBS["do_phase_c"]:
                dummy = pc.tile([P, FREE], f32)
                nc.sync.dma_start(dummy[:], attn_dram[0:P, :])
                nc.sync.dma_start(outT[0:P, :], dummy[:])
            FC = H * VD // P  # 32 feature chunks
            for pz in range(4 if KNOBS["do_phase_c"] else 0):
                psums = [pcps.tile([P, FREE], f32, tag="po",
                                   name=f"po{pz}_{i}") for i in range(8)]
                for fc in range(FC):
                    at = pc.tile([P, FREE], fr, tag="at")
                    nc.sync.dma_start(at[:],
                                      attn_dram[fc * P:(fc + 1) * P, :]
                                      .bitcast(fr))
                    wot = pc.tile([P, 8, P], fr, tag="wo")
                    nc.sync.dma_start(
                        wot[:],
                        wo[fc * P:(fc + 1) * P,
                           pz * 1024:(pz + 1) * 1024]
                        .rearrange("p (i c) -> p i c", c=P).bitcast(fr))
                    for i in range(8):
                        nc.tensor.matmul(psums[i][:], wot[:, i, :], at[:],
                                         start=(fc == 0), stop=(fc == FC - 1))
                for i in range(8):
                    osb = pc.tile([P, FREE], f32, tag="osb")
                    nc.vector.tensor_copy(osb[:], psums[i][:])
                    nc.sync.dma_start(
                        outT[(pz * 8 + i) * P:(pz * 8 + i + 1) * P, :], osb[:])

    nc.finalize()
    _CACHED[key] = nc
    return nc


def _prep_in_maps(hidden_states, cos, sin, q_a_w, q_a_ln_w, q_b_w, kv_a_w,
                  kv_a_ln_w, kv_b_w, o_w):
    hs = np.asarray(hidden_states, np.float32)
    cos = np.asarray(cos, np.float32)
    sin = np.asarray(sin, np.float32)
    rp = np.concatenate([np.arange(0, ROPE, 2), np.arange(1, ROPE, 2)])

    wqa_T = np.ascontiguousarray(np.asarray(q_a_w, np.float32).T)
    qb = (np.asarray(q_b_w, np.float32)
          * np.asarray(q_a_ln_w, np.float32)[None, :]).reshape(H, QK, QLR)
    qb = np.concatenate([qb[:, :NOPE], qb[:, NOPE:][:, rp]], axis=1)
    wqb_T = np.ascontiguousarray(qb.reshape(H * QK, QLR).T)
    kva = np.asarray(kv_a_w, np.float32).copy()
    kva[KVLR:] = kva[KVLR:][rp]
    wkva_T = np.ascontiguousarray(kva.T)
    wkvb_T = np.ascontiguousarray(
        (np.asarray(kv_b_w, np.float32)
         * np.asarray(kv_a_ln_w, np.float32)[None, :]).T)
    wo_T = np.ascontiguousarray(np.asarray(o_w, np.float32).T)
    ones = np.ones((P, P), np.float32)

    tr = np.tril(np.full((256, 256), NEG, np.float32), -1)
    pm = np.zeros((ROPE, ROPE), np.float32)
    for i_ in range(ROPE):
        pm[(i_ + ROPE // 2) % ROPE, i_] = 1.0
    in_maps = []
    for c in range(8):
        b, j = divmod(c, 4)
        qE = 256 * j            # Early query block: globals [qE, qE+256)
        qL = 1024 + 256 * j     # Late query block
        eb = np.arange(qE, qE + 256)
        lb = np.arange(qL, qL + 256)
        restA = np.arange(0, qE)
        restB = np.concatenate([np.arange(qE + 256, qL),
                                np.arange(qL + 256, S)])
        # keys: [E, restA, restB, L]; queries: [E, L]
        perm = np.concatenate([eb, restA, restB, lb])
        qtok = np.concatenate([eb, lb])
        kh_T = np.ascontiguousarray(hs[b].T[:, perm])
        qh_T = np.ascontiguousarray(hs[b].T[:, qtok])
        cq = np.ascontiguousarray(cos[b, qtok].T)
        sq_ = np.ascontiguousarray(sin[b, qtok].T)
        ck = np.ascontiguousarray(cos[b][perm].T)
        sk_ = np.ascontiguousarray(sin[b][perm].T)
        # E pass col-masks (kb 2..7): visible iff kb < 2 + qE/128
        mcE = np.full((P, 8), NEG, np.float32)
        mcE[:, :2 + qE // P] = 0.0
        # L pass col-masks (kb 0..13): visible prefix up to index 1024+qE
        mcL = np.full((P, NKB), NEG, np.float32)
        mcL[:, :(1024 + qE) // P] = 0.0
        in_maps.append({
            "kh_T": kh_T,
            "qh_T": qh_T,
            "wq_a_T": wqa_T, "wq_b_T": wqb_T,
            "wkv_a_T": wkva_T, "wkv_b_T": wkvb_T, "wo_T": wo_T,
            "cc2q": np.concatenate([cq, cq], 0),
            "ss2q": np.concatenate([-sq_, sq_], 0),
            "cc2k": np.concatenate([ck, ck], 0),
            "ss2k": np.concatenate([-sk_, sk_], 0),
            "perm64": pm,
            "mask_tril": tr,
            "mask_col_e": mcE,
            "mask_col_l": mcL,
            "ones_in": ones,
        })
    return in_maps


def kernel(**inputs) -> np.ndarray:
    nc = _build_nc()
    in_maps = _prep_in_maps(**inputs)
    res = run_bass_kernel_spmd(nc, in_maps, core_ids=list(range(8)))
    out = np.empty((B, S, HID), np.float32)
    for c in range(8):
        b, j = divmod(c, 4)
        oT = res.results[c]["out_T"]
        out[b, 256 * j:256 * (j + 1), :] = oT[:, :256].T
        out[b, 1024 + 256 * j:1024 + 256 * (j + 1), :] = oT[:, 256:].T
    return out



# revision 20
# speedup vs baseline: 4.4689x; 4.4689x over previous
"""DeepseekV2 MLA attention (B=2, S=2048, HID=4096, H=32, QK=192, VD=128)
on 8 trn2 NeuronCores — head-sharded tensor parallel.

Sharding: core c computes latents (q_a / kv_a + rope) for global token slab
c (batch c//4, quarter c%4), AllGathers the bf16 latents across all 8 cores,
then runs full causal attention for ITS 4 global heads [4c, 4c+4) over both
batches (processed sequentially). Attention outputs are AllToAll'd back to
token slabs and each core runs the full o_proj for its 512 tokens.

This removes the 4x kv_b replication of the batch-sharded layout and all
causal padding waste (each query block i only visits its 2i+2 visible key
blocks — identical static program on every core).

Precision: weights + collective transport in bf16, scores q/k in float32r,
PSUM accumulation f32, probs bf16.
"""
import sys

sys.path.insert(0, "/opt/trn_rl_repo")

import numpy as np
import ml_dtypes
import concourse.bass as bass  # noqa: F401
from concourse import bacc
import concourse.mybir as mybir
import concourse.tile as tile
from concourse.bass_utils import run_bass_kernel_spmd

# ---- problem constants (hardcoded per contract) ----
B, S, HID = 2, 2048, 4096
H, NOPE, ROPE, VD = 32, 128, 64, 128
QK = NOPE + ROPE          # 192
QLR, KVLR = 1536, 512
EPS = 1e-6
SCALE = QK ** -0.5

P = 128
TOKQ = 512                # latent tokens per core (global slab)
HL = 4                    # heads per core
NKB = S // P              # 16 key blocks per batch
NQB = S // 256            # 8 query blocks of 256
NEG = np.float32(-1e32)

bf = mybir.dt.bfloat16
fr = mybir.dt.float32r
f32 = mybir.dt.float32

_CACHED = {}

KNOBS = dict(emit_cc=True, tril_gpsimd=False, pss_bufs=2, psA_bufs=2)


def _build_nc():
    key = tuple(sorted(KNOBS.items()))
    if key in _CACHED:
        return _CACHED[key]
    nc = bacc.Bacc("TRN2", target_bir_lowering=False, debug=False, num_devices=8)

    def din(name, shape, dt=bf):
        return nc.dram_tensor(name, shape, dt, kind="ExternalInput").ap()

    hsq = din("hsq_T", [HID, TOKQ])                 # my token slab, feature-major
    wqa = din("wq_a_T", [HID, QLR])
    wqb = din("wq_b_T", [QLR, HL * QK])             # 4x nope(128) then 2x rope-pair(128)
    wkva = din("wkv_a_T", [HID, KVLR + ROPE])
    wkvb = din("wkv_b_T", [KVLR, HL * (NOPE + VD)])
    wo = din("wo_T", [H * VD, HID])
    cc2k_in = din("cc2k", [ROPE, TOKQ], f32)
    ss2k_in = din("ss2k", [ROPE, TOKQ], f32)
    cc2q_in = din("cc2q", [B, 2 * ROPE, S], f32)
    ss2q_in = din("ss2q", [B, 2 * ROPE, S], f32)
    perm_in = din("perm128", [P, P], f32)
    mtril = din("mask_tril", [256, 256], f32)
    ones_in = din("ones_f", [P, P], f32)
    onesb_in = din("ones_b", [P, P], bf)
    outT = nc.dram_tensor("out_T", [HID, TOKQ], f32, kind="ExternalOutput").ap()

    # collective buffers (DRAM). inputs Local, outputs Shared.
    agk_in = nc.dram_tensor("agk_in", [KVLR + ROPE, TOKQ], bf).ap()
    agk_out = nc.dram_tensor("agk_out", [8, KVLR + ROPE, TOKQ], bf,
                             addr_space="Shared").ap()
    agq_in = nc.dram_tensor("agq_in", [QLR, TOKQ], bf).ap()
    agq_out = nc.dram_tensor("agq_out", [8, QLR, TOKQ], bf,
                             addr_space="Shared").ap()
    a2a_in = nc.dram_tensor("a2a_in", [8, HL * VD, TOKQ], bf).ap()
    a2a_out = nc.dram_tensor("a2a_out", [8, HL * VD, TOKQ], bf).ap()

    G8 = [[0, 1, 2, 3, 4, 5, 6, 7]]
    QC = QLR // P             # 12
    KC = KVLR // P            # 4
    HC = HID // P             # 32

    wkvb4 = wkvb.rearrange("(lc p) (hh c) -> p lc hh c", p=P, c=NOPE + VD)

    def emit_cc(kind, in_ap, out_ap):
        if KNOBS["emit_cc"]:
            nc.gpsimd.collective_compute(
                kind, mybir.AluOpType.bypass, replica_groups=G8,
                ins=[in_ap.opt()], outs=[out_ap.opt()])
        else:
            # sim-only stand-in: DMA copies that preserve the dependency
            # structure (and rough byte cost) of the collective.
            n = out_ap.shape[0]
            for s in range(n):
                src = in_ap[s] if list(in_ap.shape) == list(out_ap.shape) \
                    else in_ap
                nc.gpsimd.dma_start(out_ap[s], src)

    with tile.TileContext(nc) as tc:
        with tc.tile_pool(name="persist", bufs=1) as persist:
            tril_sb = persist.tile([P, 2, 256], f32)
            perm_sb = persist.tile([P, P], fr)
            ones_fr1 = persist.tile([1, P], fr)     # row of ones (bcast stat.)
            ones_fr128 = persist.tile([P, 1], fr)   # col of ones (sums stat.)
            ones_bf128 = persist.tile([P, 1], bf)   # col of ones (denom stat.)
            eps_t = persist.tile([1, 1], f32)
            nc.vector.memset(eps_t[:], EPS)
            nc.scalar.dma_start(tril_sb[:], mtril.rearrange("(kb p) q -> p kb q", p=P))
            nc.scalar.dma_start(perm_sb[:], perm_in.bitcast(fr))
            nc.scalar.dma_start(ones_fr1[:], ones_in[0:1, :].bitcast(fr))
            nc.scalar.dma_start(ones_fr128[:], ones_in[:, 0:1].bitcast(fr))
            nc.scalar.dma_start(ones_bf128[:], onesb_in[:, 0:1])

            # ================= Phase A: my token slab's latents =================
            with tc.tile_pool(name="pa", bufs=1) as pa, \
                 tc.tile_pool(name="paw", bufs=2) as paw, \
                 tc.tile_pool(name="pasq", bufs=3) as pasq, \
                 tc.tile_pool(name="pars", bufs=4) as pars, \
                 tc.tile_pool(name="pa_ps", bufs=5, space="PSUM") as paps, \
                 tc.tile_pool(name="pa_ps2", bufs=2, space="PSUM") as paps2:
                hsq_sb = pa.tile([P, HC, TOKQ], bf)
                cc2k = pa.tile([ROPE, TOKQ], f32)
                ss2k = pa.tile([ROPE, TOKQ], f32)
                nc.scalar.dma_start(cc2k[:], cc2k_in)
                nc.scalar.dma_start(ss2k[:], ss2k_in)

                # ---- kv pass: 4 latent chunks + rope, fused normalize ----
                # interleave input/weight DMA in 8-chunk slices so the first
                # matmuls start ~4x earlier than a monolithic load
                wkv_sb = paw.tile([P, HC, KVLR + ROPE], bf, tag="wkv",
                                  bufs=1, name="wkv")
                hsq_r = hsq.rearrange("(hc p) t -> p hc t", p=P)
                wkv_r = wkva.rearrange("(hc p) m -> p hc m", p=P)
                for ch in range(8):
                    csl = slice(ch * 4, (ch + 1) * 4)
                    nc.sync.dma_start(hsq_sb[:, csl, :], hsq_r[:, csl, :])
                    nc.sync.dma_start(wkv_sb[:, csl, :], wkv_r[:, csl, :])
                kps = [paps.tile([P, TOKQ], f32, tag="pa", name=f"kv{i}")
                       for i in range(KC)]
                rotps = paps.tile([ROPE, TOKQ], f32, tag="pa", name="rot")
                sums_k = paps2.tile([1, TOKQ], f32, tag="sums", name="sums_k")
                for hc in range(HC):
                    for i in range(KC):
                        nc.tensor.matmul(
                            kps[i][:], wkv_sb[:, hc, i * P:(i + 1) * P],
                            hsq_sb[:, hc, :],
                            start=(hc == 0), stop=(hc == HC - 1))
                    nc.tensor.matmul(
                        rotps[:], wkv_sb[:, hc, KVLR:], hsq_sb[:, hc, :],
                        start=(hc == 0), stop=(hc == HC - 1))
                for i in range(KC):
                    sq = pasq.tile([P, TOKQ], fr, tag="sq", name=f"sqk{i}")
                    nc.scalar.activation(sq[:], kps[i][:],
                                         mybir.ActivationFunctionType.Square)
                    nc.tensor.matmul(sums_k[:], ones_fr128[:], sq[:],
                                     start=(i == 0), stop=(i == KC - 1))
                rsk_sq = pars.tile([1, TOKQ], f32, tag="rs", name="rsk_sq")
                nc.scalar.activation(rsk_sq[:], sums_k[:],
                                     mybir.ActivationFunctionType.Sqrt,
                                     bias=eps_t[:], scale=1.0 / KVLR)
                rsk_row = pars.tile([1, TOKQ], fr, tag="rs", name="rsk_row")
                with nc.allow_low_precision(reason="fr rms scale ~2e-4"):
                    nc.vector.reciprocal(rsk_row[:], rsk_sq[:])
                psb = paps2.tile([P, TOKQ], f32, tag="sums", name="bc_k")
                nc.tensor.matmul(psb[:], ones_fr1[:], rsk_row[:],
                                 start=True, stop=True)
                rskb = pars.tile([P, TOKQ], f32, tag="rsb", name="rskb")
                nc.vector.tensor_copy(rskb[:], psb[:])
                k_stage = pa.tile([P, KC, TOKQ], bf)
                with nc.allow_low_precision(reason="bf16 latent transport"):
                    for i in range(KC):
                        nc.vector.tensor_tensor(
                            out=k_stage[:, i, :], in0=kps[i][:], in1=rskb[:],
                            op=mybir.AluOpType.mult)
                nc.sync.dma_start(
                    agk_in[0:KVLR, :].rearrange("(lc p) t -> p lc t", p=P),
                    k_stage[:])
                # rope on k_rot (no norm): partner swap via sbuf-sbuf DMA
                kr_raw = pars.tile([ROPE, TOKQ], f32, tag="kr", name="kr_raw")
                nc.vector.tensor_copy(kr_raw[:], rotps[:])
                kr_par = pars.tile([ROPE, TOKQ], f32, tag="kr", name="kr_par")
                HR = ROPE // 2
                nc.sync.dma_start(kr_par[:HR, :], kr_raw[HR:, :])
                nc.sync.dma_start(kr_par[HR:, :], kr_raw[:HR, :])
                t1 = pars.tile([ROPE, TOKQ], f32, tag="kr", name="kr_t1")
                nc.vector.tensor_tensor(out=t1[:], in0=kr_raw[:], in1=cc2k[:],
                                        op=mybir.AluOpType.mult)
                t2 = pars.tile([ROPE, TOKQ], f32, tag="kr", name="kr_t2")
                nc.vector.tensor_tensor(out=t2[:], in0=kr_par[:], in1=ss2k[:],
                                        op=mybir.AluOpType.mult)
                krn = pars.tile([ROPE, TOKQ], bf, tag="kr", name="krn")
                with nc.allow_low_precision(reason="bf16 latent transport"):
                    nc.vector.tensor_tensor(out=krn[:], in0=t1[:], in1=t2[:],
                                            op=mybir.AluOpType.add)
                nc.sync.dma_start(agk_in[KVLR:, :], krn[:])
                emit_cc("AllGather", agk_in, agk_out)

                # ---- q passes: 12 chunks in 3 groups of 4 ----
                q_stage = pa.tile([P, QC, TOKQ], bf)
                sums_q = paps2.tile([1, TOKQ], f32, tag="sums", name="sums_q")
                for grp in range(3):
                    wq_sb = paw.tile([P, HC, 4 * P], bf, tag="w",
                                     name=f"wq{grp}")
                    nc.sync.dma_start(
                        wq_sb[:],
                        wqa[:, grp * 4 * P:(grp + 1) * 4 * P]
                        .rearrange("(hc p) m -> p hc m", p=P))
                    qps = [paps.tile([P, TOKQ], f32, tag="pa",
                                     name=f"q{grp}_{i}") for i in range(4)]
                    for hc in range(HC):
                        for i in range(4):
                            nc.tensor.matmul(
                                qps[i][:], wq_sb[:, hc, i * P:(i + 1) * P],
                                hsq_sb[:, hc, :],
                                start=(hc == 0), stop=(hc == HC - 1))
                    for i in range(4):
                        ob = grp * 4 + i
                        sq = pasq.tile([P, TOKQ], fr, tag="sq",
                                       name=f"sqq{ob}")
                        nc.scalar.activation(
                            sq[:], qps[i][:],
                            mybir.ActivationFunctionType.Square)
                        nc.tensor.matmul(sums_q[:], ones_fr128[:], sq[:],
                                         start=(ob == 0), stop=(ob == QC - 1))
                        with nc.allow_low_precision(reason="bf16 latent transport"):
                            nc.vector.tensor_copy(q_stage[:, ob, :], qps[i][:])
                rsq_sq = pars.tile([1, TOKQ], f32, tag="rs", name="rsq_sq")
                nc.scalar.activation(rsq_sq[:], sums_q[:],
                                     mybir.ActivationFunctionType.Sqrt,
                                     bias=eps_t[:], scale=1.0 / QLR)
                rsq_row = pars.tile([1, TOKQ], fr, tag="rs", name="rsq_row")
                with nc.allow_low_precision(reason="fr rms scale ~2e-4"):
                    nc.vector.reciprocal(rsq_row[:], rsq_sq[:])
                psbq = paps2.tile([P, TOKQ], f32, tag="sums", name="bc_q")
                nc.tensor.matmul(psbq[:], ones_fr1[:], rsq_row[:],
                                 start=True, stop=True)
                rsqb = pars.tile([P, TOKQ], f32, tag="rsb", name="rsqb")
                nc.vector.tensor_copy(rsqb[:], psbq[:])
                with nc.allow_low_precision(reason="bf16 latent transport"):
                    for ob in range(QC):
                        nc.vector.tensor_tensor(
                            out=q_stage[:, ob, :], in0=q_stage[:, ob, :],
                            in1=rsqb[:], op=mybir.AluOpType.mult)
                nc.scalar.dma_start(
                    agq_in.rearrange("(lc p) t -> p lc t", p=P), q_stage[:])
                emit_cc("AllGather", agq_in, agq_out)

            # ================= Phase B: 4 heads x 2 batches =================
            with tc.tile_pool(name="lat", bufs=1) as lat, \
                 tc.tile_pool(name="hb", bufs=2) as hb, \
                 tc.tile_pool(name="hb1", bufs=1) as hb1, \
                 tc.tile_pool(name="hbq", bufs=2) as hbq, \
                 tc.tile_pool(name="hb4", bufs=4) as hb4, \
                 tc.tile_pool(name="wb", bufs=2) as wb, \
                 tc.tile_pool(name="rt", bufs=2) as rt, \
                 tc.tile_pool(name="ps_A", bufs=KNOBS["psA_bufs"], space="PSUM") as psA, \
                 tc.tile_pool(name="ps_s", bufs=KNOBS["pss_bufs"], space="PSUM") as ps_s, \
                 tc.tile_pool(name="ps_o", bufs=2, space="PSUM") as ps_o, \
                 tc.tile_pool(name="ps_d", bufs=2, space="PSUM") as ps_d:
                for bb in range(B):
                    # this batch's gathered latents (tag ring 1 => batches
                    # serialize on the same SBUF)
                    q_lat = lat.tile([P, QC, S], bf, tag="qlat",
                                     name=f"qlat{bb}")
                    k_lat = lat.tile([P, KC, S], bf, tag="klat",
                                     name=f"klat{bb}")
                    # k_rot duplicated on partitions 64:128 so odd heads'
                    # q_rot slice (base partition 64) has a matching
                    # stationary base.
                    krot_bf = lat.tile([P, S], bf, tag="krbf",
                                       name=f"krbf{bb}")
                    k_rot = lat.tile([P, S], fr, tag="krot",
                                     name=f"krot{bb}")
                    cc2q = lat.tile([2 * ROPE, S], f32, tag="ccq",
                                    name=f"ccq{bb}")
                    ss2q = lat.tile([2 * ROPE, S], f32, tag="ssq",
                                    name=f"ssq{bb}")
                    # k-side loads on the sync ring (ready first)...
                    for gp in range(4):
                        sl = slice(gp * TOKQ, (gp + 1) * TOKQ)
                        nc.sync.dma_start(
                            k_lat[:, :, sl],
                            agk_out[4 * bb + gp, 0:KVLR, :]
                            .rearrange("(lc p) t -> p lc t", p=P))
                        nc.sync.dma_start(krot_bf[0:ROPE, sl],
                                          agk_out[4 * bb + gp, KVLR:, :])
                    nc.sync.dma_start(krot_bf[ROPE:, :], krot_bf[0:ROPE, :])
                    nc.vector.tensor_copy(k_rot[:], krot_bf[:])
                    # ...q-side loads on the gpsimd ring so they don't
                    # head-block the k-path weight DMAs while the q
                    # AllGather is still in flight
                    for gp in range(4):
                        sl = slice(gp * TOKQ, (gp + 1) * TOKQ)
                        nc.gpsimd.dma_start(
                            q_lat[:, :, sl],
                            agq_out[4 * bb + gp, :, :]
                            .rearrange("(lc p) t -> p lc t", p=P))
                    nc.gpsimd.dma_start(cc2q[:], cc2q_in[bb])
                    nc.gpsimd.dma_start(ss2q[:], ss2q_in[bb])

                    # v for all 4 local heads, token-major
                    vg = hb1.tile([P, NKB, HL * VD], bf, tag="vg",
                                  name=f"vg{bb}")
                    wv = wb.tile([P, KC, HL, VD], bf, tag="wv", name=f"wv{bb}")
                    for lc in range(KC):
                        nc.sync.dma_start(wv[:, lc], wkvb4[:, lc, :, NOPE:])
                    for tb in range(NKB):
                        psv = psA.tile([P, HL * VD], f32, tag="A",
                                       name=f"v{bb}_{tb}")
                        for lc in range(KC):
                            nc.tensor.matmul(
                                psv[:], k_lat[:, lc, tb * P:(tb + 1) * P],
                                wv[:, lc].rearrange("p a b -> p (a b)"),
                                start=(lc == 0), stop=(lc == KC - 1))
                        with nc.allow_low_precision(reason="bf16 v"):
                            nc.vector.tensor_copy(vg[:, tb, :], psv[:])

                    for h in range(HL):
                        pr, side = divmod(h, 2)
                        # k_pass for this head [NOPE, S] (fr for scores)
                        wk = wb.tile([P, KC, NOPE], bf, tag="wk",
                                     name=f"wk{bb}_{h}")
                        nc.sync.dma_start(wk[:], wkvb4[:, :, h, :NOPE])
                        kh_sb = hb.tile([NOPE, 4, TOKQ], fr, tag="khead",
                                        name=f"kh{bb}_{h}")
                        for tt in range(4):
                            psk = psA.tile([NOPE, TOKQ], f32, tag="A",
                                           name=f"k{bb}_{h}_{tt}")
                            for lc in range(KC):
                                nc.tensor.matmul(
                                    psk[:], wk[:, lc, :],
                                    k_lat[:, lc, tt * TOKQ:(tt + 1) * TOKQ],
                                    start=(lc == 0), stop=(lc == KC - 1))
                            nc.vector.tensor_copy(kh_sb[:, tt, :], psk[:])
                        # q nope for this head
                        wqn = wb.tile([P, QC, NOPE], bf, tag="wqn",
                                      name=f"wqn{bb}_{h}")
                        nc.sync.dma_start(
                            wqn[:],
                            wqb[:, h * NOPE:(h + 1) * NOPE]
                            .rearrange("(lc p) m -> p lc m", p=P))
                        q_pass = hbq.tile([NOPE, 4, TOKQ], fr, tag="qpass",
                                          name=f"qp{bb}_{h}")
                        for tt in range(4):
                            psq = psA.tile([NOPE, TOKQ], f32, tag="A",
                                           name=f"qn{bb}_{h}_{tt}")
                            for lc in range(QC):
                                nc.tensor.matmul(
                                    psq[:], wqn[:, lc, :],
                                    q_lat[:, lc, tt * TOKQ:(tt + 1) * TOKQ],
                                    start=(lc == 0), stop=(lc == QC - 1))
                            nc.vector.tensor_copy(q_pass[:, tt, :], psq[:])
                        # q rope for head PAIR (computed at even h)
                        if side == 0:
                            wqr = wb.tile([P, QC, P], bf, tag="wqr",
                                          name=f"wqr{bb}_{pr}")
                            nc.sync.dma_start(
                                wqr[:],
                                wqb[:, HL * NOPE + pr * P:HL * NOPE + (pr + 1) * P]
                                .rearrange("(lc p) m -> p lc m", p=P))
                            qr_pair = hbq.tile([P, 4, TOKQ], fr, tag="qr",
                                               bufs=1, name=f"qr{bb}_{pr}")
                            q_rot = hbq.tile([P, 4, TOKQ], fr, tag="qrot",
                                             bufs=1, name=f"qrot{bb}_{pr}")
                            for tt in range(4):
                                psr = psA.tile([P, TOKQ], f32, tag="A",
                                               name=f"qr{bb}_{pr}_{tt}")
                                for lc in range(QC):
                                    nc.tensor.matmul(
                                        psr[:], wqr[:, lc, :],
                                        q_lat[:, lc, tt * TOKQ:(tt + 1) * TOKQ],
                                        start=(lc == 0), stop=(lc == QC - 1))
                                nc.vector.tensor_copy(qr_pair[:, tt, :], psr[:])
                                psp = psA.tile([P, TOKQ], f32, tag="A",
                                               name=f"qperm{bb}_{pr}_{tt}")
                                nc.tensor.matmul(psp[:], perm_sb[:],
                                                 qr_pair[:, tt, :],
                                                 start=True, stop=True)
                                sl = slice(tt * TOKQ, (tt + 1) * TOKQ)
                                r1 = rt.tile([P, TOKQ], f32, tag="r",
                                             name=f"r1_{bb}_{pr}_{tt}")
                                nc.vector.tensor_tensor(
                                    out=r1[:], in0=qr_pair[:, tt, :],
                                    in1=cc2q[:, sl], op=mybir.AluOpType.mult)
                                r2 = rt.tile([P, TOKQ], f32, tag="r",
                                             name=f"r2_{bb}_{pr}_{tt}")
                                nc.vector.tensor_tensor(
                                    out=r2[:], in0=psp[:], in1=ss2q[:, sl],
                                    op=mybir.AluOpType.mult)
                                nc.vector.tensor_tensor(
                                    out=q_rot[:, tt, :], in0=r1[:], in1=r2[:],
                                    op=mybir.AluOpType.add)
                        qro = (h % 2) * ROPE   # partition offset in pair tile

                        # ---- causal attention: block i sees kb 0..2i+1 ----
                        attn_sb = hb.tile([VD, S], bf, tag="attn",
                                          name=f"at{bb}_{h}")
                        for ip in range(NQB // 2):   # i pairs (2ip, 2ip+1)
                            pso = ps_o.tile([VD, 2 * 256], f32, tag="o",
                                            name=f"o{bb}_{h}_{ip}")
                            psd = ps_d.tile([1, 2 * 256], f32, tag="d",
                                            name=f"d{bb}_{h}_{ip}")
                            for ih in range(2):
                                i = 2 * ip + ih
                                tt, half = divmod(i, 2)
                                qslt = slice(half * 256, (half + 1) * 256)
                                osl = slice(ih * 256, (ih + 1) * 256)
                                nkb = 2 * i + 2

                                def emit_dp(probs, kb, osl=osl, nkb=nkb, h=h):
                                    nc.tensor.matmul(
                                        psd[:, osl], ones_bf128[:], probs[:],
                                        start=(kb == 0), stop=(kb == nkb - 1))
                                    nc.tensor.matmul(
                                        pso[:, osl],
                                        vg[:, kb, h * VD:(h + 1) * VD],
                                        probs[:],
                                        start=(kb == 0), stop=(kb == nkb - 1))

                                pend = None
                                for kp in range(i + 1):
                                    pss = ps_s.tile([P, 512], f32, tag="s",
                                                    name=f"s{bb}_{h}_{i}_{kp}")
                                    for kh in range(2):
                                        kb = 2 * kp + kh
                                        csl = slice(kh * 256, (kh + 1) * 256)
                                        kt, ko = divmod(kb, 4)
                                        nc.tensor.matmul(
                                            pss[:, csl],
                                            kh_sb[:, kt, ko * P:(ko + 1) * P],
                                            q_pass[:, tt, qslt],
                                            start=True, stop=False)
                                        nc.tensor.matmul(
                                            pss[:, csl],
                                            k_rot[qro:qro + ROPE,
                                                  kb * P:(kb + 1) * P],
                                            q_rot[qro:qro + ROPE, tt, qslt],
                                            start=False, stop=True)
                                        if kp == i:
                                            eng = (nc.gpsimd
                                                   if KNOBS["tril_gpsimd"]
                                                   else nc.vector)
                                            eng.tensor_tensor(
                                                out=pss[:, csl],
                                                in0=pss[:, csl],
                                                in1=tril_sb[:, kh, :],
                                                op=mybir.AluOpType.add)
                                        probs = hb4.tile([P, 256], bf,
                                                         tag="probs")
                                        with nc.allow_low_precision(reason="bf16 probs"):
                                            nc.scalar.activation(
                                                probs[:], pss[:, csl],
                                                mybir.ActivationFunctionType.Exp,
                                                scale=SCALE)
                                        # psd/pso for the PREVIOUS kb — keeps
                                        # the exp chain off PE's critical path
                                        if pend is not None:
                                            emit_dp(*pend)
                                        pend = (probs, kb)
                                emit_dp(*pend)
                            # normalize pair (queries [2ip*256, (2ip+2)*256))
                            rec = hb4.tile([1, 512], fr, tag="rec", bufs=2)
                            with nc.allow_low_precision(reason="fr denom ~2e-4"):
                                nc.vector.reciprocal(rec[:], psd[:])
                            psb2 = psA.tile([P, 512], f32, tag="A",
                                            name=f"bc{bb}_{h}_{ip}")
                            nc.tensor.matmul(psb2[:], ones_fr1[:], rec[:],
                                             start=True, stop=True)
                            recb = hb4.tile([P, 512], f32, tag="recb", bufs=2)
                            nc.vector.tensor_copy(recb[:], psb2[:])
                            asl = slice(ip * 512, (ip + 1) * 512)
                            with nc.allow_low_precision(reason="bf16 attn"):
                                nc.vector.tensor_tensor(
                                    out=attn_sb[:, asl], in0=pso[:],
                                    in1=recb[:VD, :], op=mybir.AluOpType.mult)
                        # ship to a2a slabs (batch bb quarters)
                        for qtr in range(4):
                            nc.sync.dma_start(
                                a2a_in[4 * bb + qtr,
                                       h * VD:(h + 1) * VD, :],
                                attn_sb[:, qtr * TOKQ:(qtr + 1) * TOKQ])
                emit_cc("AllToAll", a2a_in, a2a_out)

        # ================= Phase C: o_proj on my token slab =================
        with tc.tile_pool(name="pc", bufs=3) as pc, \
             tc.tile_pool(name="pc_ps", bufs=8, space="PSUM") as pcps:
            for pz in range(4):
                psums = [pcps.tile([P, TOKQ], f32, tag="po",
                                   name=f"po{pz}_{i}") for i in range(8)]
                for fc in range(H):   # global head
                    at = pc.tile([P, TOKQ], bf, tag="at")
                    nc.sync.dma_start(
                        at[:],
                        a2a_out[fc // HL,
                                (fc % HL) * VD:(fc % HL + 1) * VD, :])
                    wot = pc.tile([P, 8, P], bf, tag="wo")
                    nc.sync.dma_start(
                        wot[:],
                        wo[fc * P:(fc + 1) * P, pz * 1024:(pz + 1) * 1024]
                        .rearrange("p (i c) -> p i c", c=P))
                    for i in range(8):
                        nc.tensor.matmul(psums[i][:], wot[:, i, :], at[:],
                                         start=(fc == 0), stop=(fc == H - 1))
                for i in range(8):
                    osb = pc.tile([P, TOKQ], f32, tag="osb")
                    nc.vector.tensor_copy(osb[:], psums[i][:])
                    nc.sync.dma_start(
                        outT[(pz * 8 + i) * P:(pz * 8 + i + 1) * P, :], osb[:])

    nc.finalize()
    _CACHED[key] = nc
    return nc


def _prep_in_maps(hidden_states, cos, sin, q_a_w, q_a_ln_w, q_b_w, kv_a_w,
                  kv_a_ln_w, kv_b_w, o_w):
    bft = ml_dtypes.bfloat16
    hs = np.asarray(hidden_states, np.float32)
    cos = np.asarray(cos, np.float32)
    sin = np.asarray(sin, np.float32)
    rp = np.concatenate([np.arange(0, ROPE, 2), np.arange(1, ROPE, 2)])

    wqa_T = np.ascontiguousarray(np.asarray(q_a_w, np.float32).T).astype(bft)
    qb = (np.asarray(q_b_w, np.float32)
          * np.asarray(q_a_ln_w, np.float32)[None, :]).reshape(H, QK, QLR)
    qb = np.concatenate([qb[:, :NOPE], qb[:, NOPE:][:, rp]], axis=1)
    kva = np.asarray(kv_a_w, np.float32).copy()
    kva[KVLR:] = kva[KVLR:][rp]
    wkva_T = np.ascontiguousarray(kva.T).astype(bft)
    kvb = (np.asarray(kv_b_w, np.float32)
           * np.asarray(kv_a_ln_w, np.float32)[None, :]).reshape(
               H, NOPE + VD, KVLR)
    wo_T = np.ascontiguousarray(np.asarray(o_w, np.float32).T).astype(bft)
    ones_f = np.ones((P, P), np.float32)
    ones_b = np.ones((P, P), bft)
    tr = np.tril(np.full((256, 256), NEG, np.float32), -1)
    pm64 = np.zeros((ROPE, ROPE), np.float32)
    for i_ in range(ROPE):
        pm64[(i_ + ROPE // 2) % ROPE, i_] = 1.0
    pm128 = np.zeros((P, P), np.float32)
    pm128[:ROPE, :ROPE] = pm64
    pm128[ROPE:, ROPE:] = pm64

    # q-side trig for both batches (rows duplicated for head pairs)
    cc2q = np.empty((B, 2 * ROPE, S), np.float32)
    ss2q = np.empty((B, 2 * ROPE, S), np.float32)
    for bb in range(B):
        cq = np.ascontiguousarray(cos[bb].T)     # [32, S]
        sq_ = np.ascontiguousarray(sin[bb].T)
        cc2q[bb] = np.concatenate([cq, cq, cq, cq], 0)
        ss2q[bb] = np.concatenate([-sq_, sq_, -sq_, sq_], 0)

    in_maps = []
    for c in range(8):
        bL, g = divmod(c, 4)
        tsl = slice(TOKQ * g, TOKQ * (g + 1))
        hsq_T = np.ascontiguousarray(hs[bL].T[:, tsl]).astype(bft)
        ck = np.ascontiguousarray(cos[bL, tsl].T)
        sk = np.ascontiguousarray(sin[bL, tsl].T)
        cc2k = np.concatenate([ck, ck], 0)
        ss2k = np.concatenate([-sk, sk], 0)
        # wq_b for my 4 heads: 4 nope blocks then 2 rope-pair blocks
        hsel = qb[4 * c:4 * c + 4]               # [4, QK, QLR]
        cols = [hsel[j, :NOPE].T for j in range(HL)]
        for pr2 in range(HL // 2):
            cols.append(np.concatenate(
                [hsel[2 * pr2, NOPE:], hsel[2 * pr2 + 1, NOPE:]], 0).T)
        wqb_T = np.ascontiguousarray(np.concatenate(cols, 1)).astype(bft)
        wkvb_T = np.ascontiguousarray(
            kvb[4 * c:4 * c + 4].transpose(2, 0, 1).reshape(
                KVLR, HL * (NOPE + VD))).astype(bft)
        in_maps.append({
            "hsq_T": hsq_T,
            "wq_a_T": wqa_T, "wq_b_T": wqb_T,
            "wkv_a_T": wkva_T, "wkv_b_T": wkvb_T, "wo_T": wo_T,
            "cc2k": cc2k, "ss2k": ss2k,
            "cc2q": cc2q, "ss2q": ss2q,
            "perm128": pm128,
            "mask_tril": tr,
            "ones_f": ones_f,
            "ones_b": ones_b,
        })
    return in_maps


def kernel(**inputs) -> np.ndarray:
    nc = _build_nc()
    in_maps = _prep_in_maps(**inputs)
    res = run_bass_kernel_spmd(nc, in_maps, core_ids=list(range(8)))
    out = np.empty((B, S, HID), np.float32)
    for c in range(8):
        bL, g = divmod(c, 4)
        out[bL, TOKQ * g:TOKQ * (g + 1), :] = res.results[c]["out_T"].T
    return out
